# revision 28
# baseline (speedup 1.0000x reference)
"""Cross-attention (RMSNorm + QKV proj + 2D RoPE + SDPA + out-proj) on 8
Trainium2 NeuronCores.

Sharding: 8 cores = 4 batches x 2 query-halves. Each core computes the full
KV projection for its batch (duplicated across the 2 cores sharing a batch)
and attention + output projection for its 512 query rows. No collectives.

On-device layout is feature-major: activations live as [feature, seq] with
features on SBUF partitions. Host pre-transposes inputs and weights (fp16)
so every linear layer is a plain lhsT.T @ rhs PE matmul at full rate. Head
dims are de-interleaved (even rot dims then odd rot dims per head) so RoPE's
pair rotation becomes a 32-partition block swap plus two fused
(bias-add)*table multiplies against host-precomputed sin/cos tables (fp16,
sign folded into the sin rows).

Every matmul keeps the full 128x128 array busy so the PE clock-gate (HAM)
stays at full rate: attention operands are bf16 with K for both heads of a
chunk packed on the contraction dim and the per-head Q zero-padded on its
unused 64 partitions; V tiles are padded to 128 columns (ones column at 64
for the free softmax denominator, zeros above). Projections run m-outer
with all six weight chunks resident so each PSUM accumulator retires after
six back-to-back matmuls. RMSNorm's rsqrt is exp(-0.5*ln(x)) so one ACT
table set serves the whole kernel; softmax skips max-subtraction and each
exp covers a 3-bank PSUM group (N=1536). The per-head denominator
reciprocal runs straight off PSUM with a DRAM-broadcast roundtrip, and the
output projection is a dense c-outer tail.
"""

import numpy as np

B, SQ, SK, D = 4, 1024, 1024, 768
H, HD = 12, 64
DC = D // 128          # 6 feature chunks
SQL = SQ // 2          # 512 query rows per core
SKC = SK // 128        # 8 key chunks
EPS = 1e-5
NCORES = 8

_cache = {}


# ---------------------------------------------------------------------------
# compiler workarounds
# ---------------------------------------------------------------------------

def _apply_patches():
    """This walrus build allows only ONE sync-wait command per instruction.
    (a) split the Tile kernel-tail drain into one drain per waited proc;
    (b) post-process the BIR JSON, moving excess waits onto same-engine NoOps
    inserted immediately before the over-subscribed instruction."""
    import json
    import concourse.tile as tile
    import concourse.bass as cbass
    from concourse.vector_clock import ScopedClock, VectorClock

    if getattr(cbass.Bass, "_wait_split_patched", False):
        return

    def _drain_and_barrier(self, tick_clock, wait_clock):
        gc = tick_clock.global_clock
        try:
            vec = gc[None]
        except Exception:
            vec = gc
        n = len(vec)
        for p in [i for i in range(n) if vec[i] > 0]:
            sub = [0] * n
            sub[p] = vec[p]
            inst = self.nc.sync.drain()
            wait_clock.add_sem_waits(inst.ins, ScopedClock({None: VectorClock(sub)}))
        self.nc.all_engine_barrier()
        assert self.sems is not None
        popped = self.nc._tile_sem_poison_stack.pop()
        assert popped is self._sem_poison
        self.nc.clear_and_free_semaphores(list(self.sems.allocated().values()))
        self.nc.all_engine_barrier()

    tile.TileContext._drain_and_barrier = _drain_and_barrier

    def _split_waits(bir):
        for f in bir.get("functions", []):
            for blk in f.get("blocks", []):
                insts = blk.get("instructions")
                if not insts:
                    continue
                out = []
                ctr = 0
                for inst in insts:
                    si = inst.get("sync_info")
                    ow = (si or {}).get("on_wait") or []
                    if len(ow) > 1:
                        for w in ow[:-1]:
                            nop = {
                                "name": f"{inst['name']}-ws{ctr}",
                                "opcode": "NoOp",
                                "engine": inst.get("engine"),
                                "ins": [],
                                "outs": [],
                                "sync_info": {"on_wait": [w], "on_update": []},
                            }
                            if "debug" in inst:
                                nop["debug"] = inst["debug"]
                            ctr += 1
                            out.append(nop)
                        si["on_wait"] = [ow[-1]]
                    out.append(inst)
                blk["instructions"] = out
        return bir

    orig = cbass.Bass.to_json_bytes

    def to_json_bytes(self, *a, **kw):
        return json.dumps(_split_waits(json.loads(orig(self, *a, **kw)))).encode()

    cbass.Bass.to_json_bytes = to_json_bytes
    cbass.Bass._wait_split_patched = True


# ---------------------------------------------------------------------------
# device program
# ---------------------------------------------------------------------------

def _build_nc():
    import concourse.bass as bass
    import concourse.tile as tile
    import concourse.mybir as mybir

    F32 = mybir.dt.float32
    F32R = mybir.dt.float32r
    F16 = mybir.dt.float16
    BF16 = mybir.dt.bfloat16
    AF = mybir.ActivationFunctionType
    ALU = mybir.AluOpType

    nc = bass.Bass()

    import concourse.bass as bass_mod

    qT_d = nc.dram_tensor("qT", [128, DC * SQL], F16, kind="ExternalInput")
    kvT_d = nc.dram_tensor("kvT", [128, DC * SK], F16, kind="ExternalInput")
    sqt_d = nc.dram_tensor("sq_tbl", [D, 2, SQL], BF16, kind="ExternalInput")
    skt_d = nc.dram_tensor("sk_tbl", [D, 2, SK], BF16, kind="ExternalInput")
    wq_d = nc.dram_tensor("wqT", [128, DC * D], F16, kind="ExternalInput")
    wk_d = nc.dram_tensor("wkT", [128, DC * D], F16, kind="ExternalInput")
    wv_d = nc.dram_tensor("wvT", [128, DC * D], F16, kind="ExternalInput")
    wo_d = nc.dram_tensor("woT", [128, DC * D], BF16, kind="ExternalInput")
    bias_d = nc.dram_tensor("biasR", [128, 3 * DC], F32, kind="ExternalInput")
    ones64_d = nc.dram_tensor("ones64", [1, 128], F32R, kind="ExternalInput")
    e4_d = nc.dram_tensor("e4", [128, 256], F32R, kind="ExternalInput")
    out_d = nc.dram_tensor("outT", [D, SQL], F32, kind="ExternalOutput")


    with tile.TileContext(nc) as tc:
        import contextlib
        ctx = contextlib.ExitStack()
        with ctx:
            persist = ctx.enter_context(tc.tile_pool(name="persist", bufs=1))
            ph1 = ctx.enter_context(tc.tile_pool(name="ph1", bufs=1))
            tmp = ctx.enter_context(tc.tile_pool(name="tmp", bufs=2))
            big = ctx.enter_context(tc.tile_pool(name="big", bufs=2, space="PSUM"))
            pp = ctx.enter_context(tc.tile_pool(name="pp", bufs=2, space="PSUM"))

            # ---- persistent small tensors --------------------------------
            bias_sb = persist.tile([128, 3 * DC], F32)
            ones_sb = persist.tile([128, 128], F16)
            onesc_sb = persist.tile([128, H], BF16)
            ones64 = persist.tile([1, 128], F32R)
            e4_sb = persist.tile([128, 256], F32R)
            eps_t = persist.tile([128, 1], F32)
            nc.gpsimd.dma_start(out=bias_sb, in_=bias_d[:, :])
            nc.gpsimd.dma_start(out=ones64, in_=ones64_d[:, :])
            nc.gpsimd.dma_start(out=e4_sb, in_=e4_d[:, :])
            nc.vector.memset(ones_sb, 1.0)
            nc.gpsimd.memset(onesc_sb, 1.0)
            nc.vector.memset(eps_t, EPS)

            # ---- persistent activations ---------------------------------
            qrz = [persist.tile([128, SQL], BF16, name=f"qrz{h}") for h in range(H)]
            krot = [persist.tile([128, SK], BF16, name=f"krot{c}") for c in range(DC)]
            vp = [persist.tile([128, H, 128], BF16, name=f"vp{c}") for c in range(SKC)]
            oTn = [persist.tile([128, SQL], BF16, name=f"oTn{c}") for c in range(DC)]
            out_sb = [persist.tile([128, SQL], F32, name=f"osb{c}") for c in range(DC)]

            # zero the unused halves/padding once (gpsimd engine)
            for h in range(H):
                off = 64 * (h % 2)
                nc.gpsimd.memset(qrz[h][64 - off:128 - off, :], 0.0)
            for kc in range(SKC):
                nc.gpsimd.memset(vp[kc][:, :, HD + 1:128], 0.0)

            # ---- weight + input loads (one DMA per tensor) --------------
            def wfull(dram, dt):
                t = tmp.tile([128, DC * D], dt, tag="wfull", bufs=4, name="wf")
                nc.scalar.dma_start(out=t, in_=dram[:, :])
                return t

            kvT_t = ph1.tile([128, DC * SK], F16, name="kvT")
            qT_t = ph1.tile([128, DC * SQL], F16, name="qT")
            nc.sync.dma_start(out=kvT_t, in_=kvT_d[:, :])
            nc.scalar.dma_start(out=qT_t, in_=qT_d[:, :])
            wk = wfull(wk_d, F16)
            kvT = [kvT_t[:, c * SK:(c + 1) * SK] for c in range(DC)]
            qT = [qT_t[:, c * SQL:(c + 1) * SQL] for c in range(DC)]

            # ---- RMSNorm over features (partition dim) ------------------
            ss = pp.tile([128, SQL], F32, tag="pp")
            for c in range(DC):
                sq = tmp.tile([128, SQL], F16, tag="sq", bufs=2, name="sq")
                nc.vector.tensor_mul(out=sq, in0=qT[c], in1=qT[c])
                nc.tensor.matmul(ss, ones_sb, sq, start=(c == 0), stop=(c == DC - 1))
            lnv = tmp.tile([128, SQL], F32, tag="lnv", name="lnv")
            nc.scalar.activation(out=lnv, in_=ss, func=AF.Ln, bias=eps_t,
                                 scale=1.0 / D)
            rstd = tmp.tile([128, SQL], F32, tag="rstd", name="rstd")
            nc.scalar.activation(out=rstd, in_=lnv, func=AF.Exp, scale=-0.5)
            for c in range(DC):
                nc.vector.tensor_mul(out=qT[c], in0=qT[c], in1=rstd)

            # ---- projection + RoPE (m-outer, resident weights) ----------
            def proj_rope(wt, xs, xsl, bcol, tbl_dram, stride2, coff,
                          dmae, emit_rot):
                pk = [big.tile([128, 3 * SQL], F32, tag="big", name=f"pk{j}")
                      for j in range(2)]
                for m in range(DC):
                    j, mm = divmod(m, 3)
                    sl = pk[j][:, mm * SQL:(mm + 1) * SQL]
                    for c in range(DC):
                        nc.tensor.matmul(
                            sl, wt[:, c * D + m * 128:c * D + (m + 1) * 128],
                            xs[c][:, xsl],
                            start=(c == 0), stop=(c == DC - 1))
                    tbl = tmp.tile([128, 2 * SQL], BF16, tag="tbl", bufs=4,
                                   name="tbl")
                    t0 = tbl_dram[:, :, :]
                    nc.sync.dma_start(
                        out=tbl.rearrange("p (t s) -> p t s", t=2),
                        in_=bass_mod.AP(
                            tensor=t0.tensor,
                            offset=m * 128 * 2 * stride2 + coff,
                            ap=[[2 * stride2, 128], [stride2, 2], [1, SQL]]))
                    sin_t, cos_t = tbl[:, 0:SQL], tbl[:, SQL:2 * SQL]
                    # biased projection to SBUF, then a 32-block swapped copy
                    kp = tmp.tile([128, SQL], BF16, tag="kp", bufs=4, name="kp")
                    nc.scalar.activation(
                        out=kp, in_=sl, func=AF.Identity,
                        bias=bias_sb[:, bcol + m:bcol + m + 1])
                    t1 = tmp.tile([128, SQL], BF16, tag="sw", bufs=4, name="t1")
                    for base in (0, 64):
                        dmae.dma_start(out=t1[base:base + 32, :],
                                       in_=kp[base + 32:base + 64, :])
                        dmae.dma_start(out=t1[base + 32:base + 64, :],
                                       in_=kp[base:base + 32, :])
                    nc.vector.tensor_mul(out=t1, in0=t1, in1=sin_t)
                    emit_rot(m, kp, t1, cos_t)

            def rot_k(hs):
                def emit(m, kp, t1, cos_t):
                    dst = krot[m][:, hs]
                    nc.vector.tensor_mul(out=dst, in0=kp, in1=cos_t)
                    nc.vector.tensor_add(out=dst, in0=dst, in1=t1)
                return emit

            def rot_q(m, kp, t1, cos_t):
                for h, pr in ((2 * m, slice(0, 64)), (2 * m + 1, slice(64, 128))):
                    dst = qrz[h][pr, :]
                    nc.vector.tensor_mul(out=dst, in0=kp[pr, :],
                                         in1=cos_t[pr, :])
                    nc.vector.tensor_add(out=dst, in0=dst, in1=t1[pr, :])

            wq = wfull(wq_d, F16)
            wv = wfull(wv_d, F16)
            proj_rope(wk, kvT, slice(0, SQL), DC, skt_d, SK,
                      0, nc.gpsimd, rot_k(slice(0, SQL)))
            proj_rope(wk, kvT, slice(SQL, SK), DC, skt_d, SK,
                      SQL, nc.scalar, rot_k(slice(SQL, SK)))
            proj_rope(wq, qT, slice(0, SQL), 0, sqt_d, SQL,
                      0, nc.gpsimd, rot_q)

            # ---- V projection (row-major, ones column at 64) ------------
            for kc in range(SKC):
                ksl = slice(kc * 128, (kc + 1) * 128)
                pv = big.tile([128, 3 * SQL], F32, tag="big", name="pv")
                for c in range(DC):
                    nc.tensor.matmul(pv[:, 0:512], kvT[c][:, ksl],
                                     wv[:, c * D:c * D + 512],
                                     start=(c == 0), stop=(c == DC - 1))
                    nc.tensor.matmul(pv[:, 512:768], kvT[c][:, ksl],
                                     wv[:, c * D + 512:c * D + 768],
                                     start=(c == 0), stop=(c == DC - 1))
                nc.vector.tensor_copy(
                    out=vp[kc][:, 0:8, 0:HD],
                    in_=pv[:, 0:512].rearrange("p (h d) -> p h d", h=8))
                nc.vector.tensor_copy(
                    out=vp[kc][:, 8:12, 0:HD],
                    in_=pv[:, 512:768].rearrange("p (h d) -> p h d", h=4))
                nc.gpsimd.tensor_copy(out=vp[kc][:, :, HD], in_=onesc_sb)

            # ---- attention + pipelined per-head normalization -----------
            GROUPS = [(0, 3), (3, 6), (6, 8)]
            den4 = None
            for h in range(H):
                mh, off = h // 2, 64 * (h % 2)
                if h % 4 == 0:
                    den4 = tmp.tile([128, SQL], F32, tag="den4", bufs=2,
                                    name="den4")
                    nc.gpsimd.memset(den4, 1.0)
                po = pp.tile([128, SQL], F32, tag="pp", name="po")
                for k0, k1 in GROUPS:
                    w = (k1 - k0) * SQL
                    sc = big.tile([128, 3 * SQL], F32, tag="big", name="sc")
                    for i, kc in enumerate(range(k0, k1)):
                        nc.tensor.matmul(
                            sc[:, i * SQL:(i + 1) * SQL],
                            krot[mh][:, kc * 128:(kc + 1) * 128],
                            qrz[h], start=True, stop=True)
                    ex = tmp.tile([128, 3 * SQL], BF16, tag="ex", bufs=3,
                                  name="ex")
                    nc.scalar.activation(out=ex[:, :w], in_=sc[:, :w],
                                         func=AF.Exp, scale=1.0 / 8.0)
                    for i, kc in enumerate(range(k0, k1)):
                        nc.tensor.matmul(po, vp[kc][:, h, :],
                                         ex[:, i * SQL:(i + 1) * SQL],
                                         start=(kc == 0), stop=(kc == SKC - 1))
                # stash unnormalized O + the denominator row; po retires fast
                b4 = 32 * (h % 4)
                nc.vector.tensor_copy(out=den4[b4:b4 + 1, :],
                                      in_=po[64:65, :])
                nc.vector.tensor_copy(out=oTn[mh][off:off + 64, :],
                                      in_=po[0:64, :])
                if h % 4 == 3:
                    # batched reciprocal on ACT (exp(-ln x), free-dim bound),
                    # then per-pair partition-broadcast via a PE matmul
                    ln4 = tmp.tile([128, SQL], F32, tag="ln4", bufs=2,
                                   name="ln4")
                    nc.scalar.activation(out=ln4, in_=den4, func=AF.Ln)
                    rd4 = tmp.tile([128, SQL], F32R, tag="rd4", bufs=2,
                                   name="rd4")
                    nc.scalar.activation(out=rd4, in_=ln4, func=AF.Exp,
                                         scale=-1.0)
                    for p in range(2):
                        mh2 = (h // 4) * 2 + p
                        rbb = pp.tile([128, SQL], F32, tag="pp", name="rbb")
                        nc.tensor.matmul(rbb, e4_sb[:, p * 128:(p + 1) * 128],
                                         rd4, start=True, stop=True)
                        rbs = tmp.tile([128, SQL], F32, tag="rb", bufs=2,
                                       name="rbs")
                        nc.vector.tensor_copy(out=rbs, in_=rbb)
                        nc.vector.tensor_mul(out=oTn[mh2], in0=oTn[mh2],
                                             in1=rbs)

            # ---- output projection tail (c-outer, 2 psum accumulators) --
            wo = wfull(wo_d, BF16)
            po6 = [big.tile([128, 3 * SQL], F32, tag="big", name=f"po6_{j}")
                   for j in range(2)]
            for c in range(DC):
                for j in range(2):
                    for mm in range(3):
                        m = 3 * j + mm
                        nc.tensor.matmul(
                            po6[j][:, mm * SQL:(mm + 1) * SQL],
                            wo[:, c * D + m * 128:c * D + (m + 1) * 128],
                            oTn[c], start=(c == 0), stop=(c == DC - 1))
            for m in range(DC):
                j, mm = divmod(m, 3)
                nc.vector.tensor_scalar_add(
                    out=out_sb[m], in0=po6[j][:, mm * SQL:(mm + 1) * SQL],
                    scalar1=bias_sb[:, 2 * DC + m:2 * DC + m + 1])
                nc.sync.dma_start(out=out_d[m * 128:(m + 1) * 128, :],
                                  in_=out_sb[m])

    return nc


# ---------------------------------------------------------------------------
# host wrapper
# ---------------------------------------------------------------------------

def _rope_tables(pos, freqs):
    """pos [S,2] int, freqs [2,H,32] -> signed-sin and cos tables [D,S] fp16,
    rows in de-interleaved head-dim order (pair j at rows h*64+j / h*64+32+j,
    first-half sin rows negated so rot = x*cos + swap(x)*sin)."""
    ang = np.einsum('sd,dhj->hjs', pos.astype(np.float64),
                    freqs.astype(np.float64))          # [H,32,S]
    s, c = np.sin(ang), np.cos(ang)
    sin_full = np.concatenate([-s, s], axis=1).reshape(D, -1)
    cos_full = np.concatenate([c, c], axis=1).reshape(D, -1)
    import ml_dtypes
    bf = ml_dtypes.bfloat16
    return sin_full.astype(bf), cos_full.astype(bf)


def kernel(q, kv, posq, posk, w_norm, w_q, b_q, w_kv, b_kv, w_out, b_out, freqs):
    _apply_patches()
    from concourse.bass_utils import run_bass_kernel_spmd
    import ml_dtypes

    bf16 = ml_dtypes.bfloat16

    q = np.asarray(q, np.float32)
    kv = np.asarray(kv, np.float32)
    posq_np = np.asarray(posq)
    posk_np = np.asarray(posk)
    w_norm = np.asarray(w_norm, np.float32)
    w_q = np.asarray(w_q, np.float32)
    b_q = np.asarray(b_q, np.float32)
    w_kv = np.asarray(w_kv, np.float32)
    b_kv = np.asarray(b_kv, np.float32)
    w_out = np.asarray(w_out, np.float32)
    b_out = np.asarray(b_out, np.float32)
    freqs = np.asarray(freqs, np.float32)

    # de-interleave head dims: new j<32 -> old 2j (even), j>=32 -> old 2(j-32)+1
    perm = np.empty(D, np.int64)
    for h in range(H):
        for j in range(HD):
            perm[h * HD + j] = h * HD + (2 * j if j < 32 else 2 * (j - 32) + 1)

    def sblay(w, dt):
        # [D, W] feature-major -> [128, DC*W] SBUF tile layout
        Wd = w.shape[1]
        return np.ascontiguousarray(
            w.reshape(DC, 128, Wd).transpose(1, 0, 2).reshape(128, DC * Wd)
        ).astype(dt)

    wqT = sblay((w_q[perm, :] * w_norm[None, :]).T, np.float16)
    wkT = sblay(w_kv[:D][perm, :].T, np.float16)
    wvT = sblay(w_kv[D:].T, np.float16)
    woT = sblay(w_out.T, bf16)
    bqR = np.ascontiguousarray(b_q[perm].reshape(DC, 128).T)
    bkR = np.ascontiguousarray(b_kv[:D][perm].reshape(DC, 128).T)
    bo_eff = b_out + w_out @ b_kv[D:]          # fold V bias (softmax sums to 1)
    boR = np.ascontiguousarray(bo_eff.reshape(DC, 128).T)

    biasR = np.ascontiguousarray(
        np.concatenate([bqR, bkR, boR], axis=1)).astype(np.float32)
    e4 = np.zeros((128, 256), np.float32)
    e4[0, 0:64] = 1.0
    e4[32, 64:128] = 1.0
    e4[64, 128:192] = 1.0
    e4[96, 192:256] = 1.0

    if "nc" not in _cache:
        _cache["nc"] = _build_nc()
    nc = _cache["nc"]

    in_maps = []
    for core in range(NCORES):
        b, half = core // 2, core % 2
        qs = slice(half * SQL, (half + 1) * SQL)
        sinq, cosq = _rope_tables(posq_np[b], freqs)
        sink, cosk = _rope_tables(posk_np[b], freqs)
        in_maps.append({
            "qT": sblay(q[b, qs, :].T, np.float16),
            "kvT": sblay(kv[b].T, np.float16),
            "sq_tbl": np.ascontiguousarray(
                np.stack([sinq[:, qs], cosq[:, qs]], axis=1)),
            "sk_tbl": np.ascontiguousarray(np.stack([sink, cosk], axis=1)),
            "wqT": wqT, "wkT": wkT, "wvT": wvT, "woT": woT,
            "biasR": biasR, "ones64": np.ones((1, 128), np.float32),
            "e4": e4,
        })

    res = run_bass_kernel_spmd(nc, in_maps, core_ids=list(range(NCORES)))
    kernel._last_result = res

    out = np.empty((B, SQ, D), np.float32)
    for core in range(NCORES):
        b, half = core // 2, core % 2
        out[b, half * SQL:(half + 1) * SQL, :] = res.results[core]["outT"].T
    return out


# revision 29
# speedup vs baseline: 1.2153x; 1.2153x over previous
"""Cross-attention (RMSNorm + QKV proj + 2D RoPE + SDPA + out-proj) on 8
Trainium2 NeuronCores.

Sharding: 8 cores = 4 batches x 2 query-halves. Each core computes the full
KV projection for its batch (duplicated across the 2 cores sharing a batch)
and attention + output projection for its 512 query rows. No collectives.

On-device layout is feature-major: activations live as [feature, seq] with
features on SBUF partitions. Host pre-transposes inputs and weights (fp16)
so every linear layer is a plain lhsT.T @ rhs PE matmul at full rate. Head
dims are de-interleaved (even rot dims then odd rot dims per head) so RoPE's
pair rotation becomes a 32-partition block swap plus two fused
(bias-add)*table multiplies against host-precomputed sin/cos tables (fp16,
sign folded into the sin rows).

Every matmul keeps the full 128x128 array busy so the PE clock-gate (HAM)
stays at full rate: attention operands are bf16 with K for both heads of a
chunk packed on the contraction dim and the per-head Q zero-padded on its
unused 64 partitions; V tiles are padded to 128 columns (ones column at 64
for the free softmax denominator, zeros above). Projections run m-outer
with all six weight chunks resident so each PSUM accumulator retires after
six back-to-back matmuls. RMSNorm's rsqrt is exp(-0.5*ln(x)) so one ACT
table set serves the whole kernel; softmax skips max-subtraction and each
exp covers a 3-bank PSUM group (N=1536). The per-head denominator
reciprocal runs straight off PSUM with a DRAM-broadcast roundtrip, and the
output projection is a dense c-outer tail.
"""

import numpy as np

B, SQ, SK, D = 4, 1024, 1024, 768
H, HD = 12, 64
DC = D // 128          # 6 feature chunks
SQL = SQ // 2          # 512 query rows per core
SKC = SK // 128        # 8 key chunks
EPS = 1e-5
NCORES = 8

_cache = {}


# ---------------------------------------------------------------------------
# compiler workarounds
# ---------------------------------------------------------------------------

def _apply_patches():
    """This walrus build allows only ONE sync-wait command per instruction.
    (a) split the Tile kernel-tail drain into one drain per waited proc;
    (b) post-process the BIR JSON, moving excess waits onto same-engine NoOps
    inserted immediately before the over-subscribed instruction."""
    import json
    import concourse.tile as tile
    import concourse.bass as cbass
    from concourse.vector_clock import ScopedClock, VectorClock

    if getattr(cbass.Bass, "_wait_split_patched", False):
        return

    def _drain_and_barrier(self, tick_clock, wait_clock):
        gc = tick_clock.global_clock
        try:
            vec = gc[None]
        except Exception:
            vec = gc
        n = len(vec)
        for p in [i for i in range(n) if vec[i] > 0]:
            sub = [0] * n
            sub[p] = vec[p]
            inst = self.nc.sync.drain()
            wait_clock.add_sem_waits(inst.ins, ScopedClock({None: VectorClock(sub)}))
        self.nc.all_engine_barrier()
        assert self.sems is not None
        popped = self.nc._tile_sem_poison_stack.pop()
        assert popped is self._sem_poison
        self.nc.clear_and_free_semaphores(list(self.sems.allocated().values()))
        self.nc.all_engine_barrier()

    tile.TileContext._drain_and_barrier = _drain_and_barrier

    def _split_waits(bir):
        for f in bir.get("functions", []):
            for blk in f.get("blocks", []):
                insts = blk.get("instructions")
                if not insts:
                    continue
                out = []
                ctr = 0
                for inst in insts:
                    si = inst.get("sync_info")
                    ow = (si or {}).get("on_wait") or []
                    if len(ow) > 1:
                        for w in ow[:-1]:
                            nop = {
                                "name": f"{inst['name']}-ws{ctr}",
                                "opcode": "NoOp",
                                "engine": inst.get("engine"),
                                "ins": [],
                                "outs": [],
                                "sync_info": {"on_wait": [w], "on_update": []},
                            }
                            if "debug" in inst:
                                nop["debug"] = inst["debug"]
                            ctr += 1
                            out.append(nop)
                        si["on_wait"] = [ow[-1]]
                    out.append(inst)
                blk["instructions"] = out
        return bir

    orig = cbass.Bass.to_json_bytes

    def to_json_bytes(self, *a, **kw):
        return json.dumps(_split_waits(json.loads(orig(self, *a, **kw)))).encode()

    cbass.Bass.to_json_bytes = to_json_bytes
    cbass.Bass._wait_split_patched = True


# ---------------------------------------------------------------------------
# device program
# ---------------------------------------------------------------------------

def _build_nc():
    import concourse.bass as bass
    import concourse.tile as tile
    import concourse.mybir as mybir

    F32 = mybir.dt.float32
    F32R = mybir.dt.float32r
    F16 = mybir.dt.float16
    BF16 = mybir.dt.bfloat16
    AF = mybir.ActivationFunctionType
    ALU = mybir.AluOpType

    nc = bass.Bass()

    import concourse.bass as bass_mod

    qT_d = nc.dram_tensor("qT", [128, DC * SQL], F16, kind="ExternalInput")
    kvT_d = nc.dram_tensor("kvT", [128, DC * SK], F16, kind="ExternalInput")
    sqt_d = nc.dram_tensor("sq_tbl", [D, 2, SQL], BF16, kind="ExternalInput")
    skt_d = nc.dram_tensor("sk_tbl", [D, 2, SK], BF16, kind="ExternalInput")
    wq_d = nc.dram_tensor("wqT", [128, DC * D], F16, kind="ExternalInput")
    wk_d = nc.dram_tensor("wkT", [128, DC * D], F16, kind="ExternalInput")
    wv_d = nc.dram_tensor("wvT", [128, DC * D], F16, kind="ExternalInput")
    wo_d = nc.dram_tensor("woT", [128, DC * D], BF16, kind="ExternalInput")
    bias_d = nc.dram_tensor("biasR", [128, 3 * DC], F32, kind="ExternalInput")
    ones64_d = nc.dram_tensor("ones64", [1, 128], F32R, kind="ExternalInput")
    e4_d = nc.dram_tensor("e4", [128, 256], F32R, kind="ExternalInput")
    psw_d = nc.dram_tensor("pswap", [128, 128], BF16, kind="ExternalInput")
    out_d = nc.dram_tensor("outT", [D, SQL], F32, kind="ExternalOutput")


    with tile.TileContext(nc) as tc:
        import contextlib
        ctx = contextlib.ExitStack()
        with ctx:
            persist = ctx.enter_context(tc.tile_pool(name="persist", bufs=1))
            ph1 = ctx.enter_context(tc.tile_pool(name="ph1", bufs=1))
            tmp = ctx.enter_context(tc.tile_pool(name="tmp", bufs=2))
            big = ctx.enter_context(tc.tile_pool(name="big", bufs=2, space="PSUM"))
            pp = ctx.enter_context(tc.tile_pool(name="pp", bufs=2, space="PSUM"))

            # ---- persistent small tensors --------------------------------
            bias_sb = persist.tile([128, 3 * DC], F32)
            ones_sb = persist.tile([128, 128], F16)
            onesc_sb = persist.tile([128, H], BF16)
            ones64 = persist.tile([1, 128], F32R)
            e4_sb = persist.tile([128, 256], F32R)
            psw_sb = persist.tile([128, 128], BF16)
            eps_t = persist.tile([128, 1], F32)
            nc.gpsimd.dma_start(out=bias_sb, in_=bias_d[:, :])
            nc.gpsimd.dma_start(out=ones64, in_=ones64_d[:, :])
            nc.gpsimd.dma_start(out=e4_sb, in_=e4_d[:, :])
            nc.gpsimd.dma_start(out=psw_sb, in_=psw_d[:, :])
            nc.vector.memset(ones_sb, 1.0)
            nc.gpsimd.memset(onesc_sb, 1.0)
            nc.vector.memset(eps_t, EPS)

            # ---- persistent activations ---------------------------------
            qrz = [persist.tile([128, SQL], BF16, name=f"qrz{h}") for h in range(H)]
            krot = [persist.tile([128, SK], BF16, name=f"krot{c}") for c in range(DC)]
            vp = [persist.tile([128, H, 128], BF16, name=f"vp{c}") for c in range(SKC)]
            oTn = [persist.tile([128, SQL], BF16, name=f"oTn{c}") for c in range(DC)]
            out_sb = [persist.tile([128, SQL], F32, name=f"osb{c}") for c in range(DC)]

            # zero the unused halves/padding once (gpsimd engine)
            for h in range(H):
                off = 64 * (h % 2)
                nc.gpsimd.memset(qrz[h][64 - off:128 - off, :], 0.0)
            for kc in range(SKC):
                nc.gpsimd.memset(vp[kc][:, :, HD + 1:128], 0.0)

            # ---- weight + input loads (one DMA per tensor) --------------
            def wfull(dram, dt):
                t = tmp.tile([128, DC * D], dt, tag="wfull", bufs=4, name="wf")
                nc.scalar.dma_start(out=t, in_=dram[:, :])
                return t

            kvT_t = ph1.tile([128, DC * SK], F16, name="kvT")
            qT_t = ph1.tile([128, DC * SQL], F16, name="qT")
            nc.sync.dma_start(out=kvT_t, in_=kvT_d[:, :])
            nc.scalar.dma_start(out=qT_t, in_=qT_d[:, :])
            wk = wfull(wk_d, F16)
            kvT = [kvT_t[:, c * SK:(c + 1) * SK] for c in range(DC)]
            qT = [qT_t[:, c * SQL:(c + 1) * SQL] for c in range(DC)]

            # ---- RMSNorm over features (partition dim) ------------------
            ss = pp.tile([128, SQL], F32, tag="pp")
            for c in range(DC):
                sq = tmp.tile([128, SQL], F16, tag="sq", bufs=2, name="sq")
                nc.vector.tensor_mul(out=sq, in0=qT[c], in1=qT[c])
                nc.tensor.matmul(ss, ones_sb, sq, start=(c == 0), stop=(c == DC - 1))
            lnv = tmp.tile([128, SQL], F32, tag="lnv", name="lnv")
            nc.scalar.activation(out=lnv, in_=ss, func=AF.Ln, bias=eps_t,
                                 scale=1.0 / D)
            rstd = tmp.tile([128, SQL], F32, tag="rstd", name="rstd")
            nc.scalar.activation(out=rstd, in_=lnv, func=AF.Exp, scale=-0.5)
            for c in range(DC):
                nc.vector.tensor_mul(out=qT[c], in0=qT[c], in1=rstd)

            # ---- projection + RoPE (m-outer, resident weights) ----------
            def proj_rope(wt, xs, xsl, bcol, tbl_dram, stride2, coff,
                          emit_rot):
                pk = [big.tile([128, 3 * SQL], F32, tag="big", name=f"pk{j}")
                      for j in range(2)]
                for m in range(DC):
                    j, mm = divmod(m, 3)
                    sl = pk[j][:, mm * SQL:(mm + 1) * SQL]
                    for c in range(DC):
                        nc.tensor.matmul(
                            sl, wt[:, c * D + m * 128:c * D + (m + 1) * 128],
                            xs[c][:, xsl],
                            start=(c == 0), stop=(c == DC - 1))
                    tbl = tmp.tile([128, 2 * SQL], BF16, tag="tbl", bufs=4,
                                   name="tbl")
                    t0 = tbl_dram[:, :, :]
                    nc.sync.dma_start(
                        out=tbl.rearrange("p (t s) -> p t s", t=2),
                        in_=bass_mod.AP(
                            tensor=t0.tensor,
                            offset=m * 128 * 2 * stride2 + coff,
                            ap=[[2 * stride2, 128], [stride2, 2], [1, SQL]]))
                    sin_t, cos_t = tbl[:, 0:SQL], tbl[:, SQL:2 * SQL]
                    # biased projection to SBUF, then a 32-block swapped copy
                    kp = tmp.tile([128, SQL], BF16, tag="kp", bufs=4, name="kp")
                    nc.scalar.activation(
                        out=kp, in_=sl, func=AF.Identity,
                        bias=bias_sb[:, bcol + m:bcol + m + 1])
                    t1p = pp.tile([128, SQL], F32, tag="pp", name="t1p")
                    nc.tensor.matmul(t1p, psw_sb, kp, start=True, stop=True)
                    t1 = tmp.tile([128, SQL], BF16, tag="sw", bufs=4, name="t1")
                    nc.vector.tensor_mul(out=t1, in0=t1p, in1=sin_t)
                    emit_rot(m, kp, t1, cos_t)

            def rot_k(hs):
                def emit(m, kp, t1, cos_t):
                    dst = krot[m][:, hs]
                    nc.vector.tensor_mul(out=dst, in0=kp, in1=cos_t)
                    nc.vector.tensor_add(out=dst, in0=dst, in1=t1)
                return emit

            def rot_q(m, kp, t1, cos_t):
                for h, pr in ((2 * m, slice(0, 64)), (2 * m + 1, slice(64, 128))):
                    dst = qrz[h][pr, :]
                    nc.vector.tensor_mul(out=dst, in0=kp[pr, :],
                                         in1=cos_t[pr, :])
                    nc.vector.tensor_add(out=dst, in0=dst, in1=t1[pr, :])

            wq = wfull(wq_d, F16)
            wv = wfull(wv_d, F16)
            proj_rope(wk, kvT, slice(0, SQL), DC, skt_d, SK,
                      0, rot_k(slice(0, SQL)))
            proj_rope(wk, kvT, slice(SQL, SK), DC, skt_d, SK,
                      SQL, rot_k(slice(SQL, SK)))
            proj_rope(wq, qT, slice(0, SQL), 0, sqt_d, SQL,
                      0, rot_q)

            # ---- V projection (row-major, ones column at 64) ------------
            for kc in range(SKC):
                ksl = slice(kc * 128, (kc + 1) * 128)
                pv = big.tile([128, 3 * SQL], F32, tag="big", name="pv")
                for c in range(DC):
                    nc.tensor.matmul(pv[:, 0:512], kvT[c][:, ksl],
                                     wv[:, c * D:c * D + 512],
                                     start=(c == 0), stop=(c == DC - 1))
                    nc.tensor.matmul(pv[:, 512:768], kvT[c][:, ksl],
                                     wv[:, c * D + 512:c * D + 768],
                                     start=(c == 0), stop=(c == DC - 1))
                nc.vector.tensor_copy(
                    out=vp[kc][:, 0:8, 0:HD],
                    in_=pv[:, 0:512].rearrange("p (h d) -> p h d", h=8))
                nc.vector.tensor_copy(
                    out=vp[kc][:, 8:12, 0:HD],
                    in_=pv[:, 512:768].rearrange("p (h d) -> p h d", h=4))
                nc.gpsimd.tensor_copy(out=vp[kc][:, :, HD], in_=onesc_sb)

            # ---- attention + pipelined per-head normalization -----------
            GROUPS = [(0, 3), (3, 6), (6, 8)]
            den4 = None
            for h in range(H):
                mh, off = h // 2, 64 * (h % 2)
                if h % 4 == 0:
                    den4 = tmp.tile([128, SQL], F32, tag="den4", bufs=2,
                                    name="den4")
                    nc.gpsimd.memset(den4, 1.0)
                po = pp.tile([128, SQL], F32, tag="pp", name="po")
                for k0, k1 in GROUPS:
                    w = (k1 - k0) * SQL
                    sc = big.tile([128, 3 * SQL], F32, tag="big", name="sc")
                    for i, kc in enumerate(range(k0, k1)):
                        nc.tensor.matmul(
                            sc[:, i * SQL:(i + 1) * SQL],
                            krot[mh][:, kc * 128:(kc + 1) * 128],
                            qrz[h], start=True, stop=True)
                    ex = tmp.tile([128, 3 * SQL], BF16, tag="ex", bufs=3,
                                  name="ex")
                    nc.scalar.activation(out=ex[:, :w], in_=sc[:, :w],
                                         func=AF.Exp, scale=1.0 / 8.0)
                    for i, kc in enumerate(range(k0, k1)):
                        nc.tensor.matmul(po, vp[kc][:, h, :],
                                         ex[:, i * SQL:(i + 1) * SQL],
                                         start=(kc == 0), stop=(kc == SKC - 1))
                # stash unnormalized O + the denominator row; po retires fast
                b4 = 32 * (h % 4)
                nc.vector.tensor_copy(out=den4[b4:b4 + 1, :],
                                      in_=po[64:65, :])
                nc.vector.tensor_copy(out=oTn[mh][off:off + 64, :],
                                      in_=po[0:64, :])
                if h % 4 == 3:
                    # batched reciprocal on ACT (exp(-ln x), free-dim bound),
                    # then per-pair partition-broadcast via a PE matmul
                    ln4 = tmp.tile([128, SQL], F32, tag="ln4", bufs=2,
                                   name="ln4")
                    nc.scalar.activation(out=ln4, in_=den4, func=AF.Ln)
                    rd4 = tmp.tile([128, SQL], F32R, tag="rd4", bufs=2,
                                   name="rd4")
                    nc.scalar.activation(out=rd4, in_=ln4, func=AF.Exp,
                                         scale=-1.0)
                    for p in range(2):
                        mh2 = (h // 4) * 2 + p
                        rbb = pp.tile([128, SQL], F32, tag="pp", name="rbb")
                        nc.tensor.matmul(rbb, e4_sb[:, p * 128:(p + 1) * 128],
                                         rd4, start=True, stop=True)
                        rbs = tmp.tile([128, SQL], F32, tag="rb", bufs=2,
                                       name="rbs")
                        nc.vector.tensor_copy(out=rbs, in_=rbb)
                        nc.vector.tensor_mul(out=oTn[mh2], in0=oTn[mh2],
                                             in1=rbs)

            # ---- output projection tail (c-outer, 2 psum accumulators) --
            wo = wfull(wo_d, BF16)
            po6 = [big.tile([128, 3 * SQL], F32, tag="big", name=f"po6_{j}")
                   for j in range(2)]
            for c in range(DC):
                for j in range(2):
                    for mm in range(3):
                        m = 3 * j + mm
                        nc.tensor.matmul(
                            po6[j][:, mm * SQL:(mm + 1) * SQL],
                            wo[:, c * D + m * 128:c * D + (m + 1) * 128],
                            oTn[c], start=(c == 0), stop=(c == DC - 1))
            for m in range(DC):
                j, mm = divmod(m, 3)
                nc.vector.tensor_scalar_add(
                    out=out_sb[m], in0=po6[j][:, mm * SQL:(mm + 1) * SQL],
                    scalar1=bias_sb[:, 2 * DC + m:2 * DC + m + 1])
                nc.sync.dma_start(out=out_d[m * 128:(m + 1) * 128, :],
                                  in_=out_sb[m])

    return nc


# ---------------------------------------------------------------------------
# host wrapper
# ---------------------------------------------------------------------------

def _rope_tables(pos, freqs):
    """pos [S,2] int, freqs [2,H,32] -> signed-sin and cos tables [D,S] fp16,
    rows in de-interleaved head-dim order (pair j at rows h*64+j / h*64+32+j,
    first-half sin rows negated so rot = x*cos + swap(x)*sin)."""
    ang = np.einsum('sd,dhj->hjs', pos.astype(np.float64),
                    freqs.astype(np.float64))          # [H,32,S]
    s, c = np.sin(ang), np.cos(ang)
    sin_full = np.concatenate([-s, s], axis=1).reshape(D, -1)
    cos_full = np.concatenate([c, c], axis=1).reshape(D, -1)
    import ml_dtypes
    bf = ml_dtypes.bfloat16
    return sin_full.astype(bf), cos_full.astype(bf)


def kernel(q, kv, posq, posk, w_norm, w_q, b_q, w_kv, b_kv, w_out, b_out, freqs):
    _apply_patches()
    from concourse.bass_utils import run_bass_kernel_spmd
    import ml_dtypes

    bf16 = ml_dtypes.bfloat16

    q = np.asarray(q, np.float32)
    kv = np.asarray(kv, np.float32)
    posq_np = np.asarray(posq)
    posk_np = np.asarray(posk)
    w_norm = np.asarray(w_norm, np.float32)
    w_q = np.asarray(w_q, np.float32)
    b_q = np.asarray(b_q, np.float32)
    w_kv = np.asarray(w_kv, np.float32)
    b_kv = np.asarray(b_kv, np.float32)
    w_out = np.asarray(w_out, np.float32)
    b_out = np.asarray(b_out, np.float32)
    freqs = np.asarray(freqs, np.float32)

    # de-interleave head dims: new j<32 -> old 2j (even), j>=32 -> old 2(j-32)+1
    perm = np.empty(D, np.int64)
    for h in range(H):
        for j in range(HD):
            perm[h * HD + j] = h * HD + (2 * j if j < 32 else 2 * (j - 32) + 1)

    def sblay(w, dt):
        # [D, W] feature-major -> [128, DC*W] SBUF tile layout
        Wd = w.shape[1]
        return np.ascontiguousarray(
            w.reshape(DC, 128, Wd).transpose(1, 0, 2).reshape(128, DC * Wd)
        ).astype(dt)

    wqT = sblay((w_q[perm, :] * w_norm[None, :]).T, np.float16)
    wkT = sblay(w_kv[:D][perm, :].T, np.float16)
    wvT = sblay(w_kv[D:].T, np.float16)
    woT = sblay(w_out.T, bf16)
    bqR = np.ascontiguousarray(b_q[perm].reshape(DC, 128).T)
    bkR = np.ascontiguousarray(b_kv[:D][perm].reshape(DC, 128).T)
    bo_eff = b_out + w_out @ b_kv[D:]          # fold V bias (softmax sums to 1)
    boR = np.ascontiguousarray(bo_eff.reshape(DC, 128).T)

    biasR = np.ascontiguousarray(
        np.concatenate([bqR, bkR, boR], axis=1)).astype(np.float32)
    psw = np.zeros((128, 128), np.float32)
    for i in range(128):
        j = (i % 64)
        partner = (i - i % 64) + (j + 32 if j < 32 else j - 32)
        psw[partner, i] = 1.0
    psw = psw.astype(bf16)
    e4 = np.zeros((128, 256), np.float32)
    e4[0, 0:64] = 1.0
    e4[32, 64:128] = 1.0
    e4[64, 128:192] = 1.0
    e4[96, 192:256] = 1.0

    if "nc" not in _cache:
        _cache["nc"] = _build_nc()
    nc = _cache["nc"]

    in_maps = []
    for core in range(NCORES):
        b, half = core // 2, core % 2
        qs = slice(half * SQL, (half + 1) * SQL)
        sinq, cosq = _rope_tables(posq_np[b], freqs)
        sink, cosk = _rope_tables(posk_np[b], freqs)
        in_maps.append({
            "qT": sblay(q[b, qs, :].T, np.float16),
            "kvT": sblay(kv[b].T, np.float16),
            "sq_tbl": np.ascontiguousarray(
                np.stack([sinq[:, qs], cosq[:, qs]], axis=1)),
            "sk_tbl": np.ascontiguousarray(np.stack([sink, cosk], axis=1)),
            "wqT": wqT, "wkT": wkT, "wvT": wvT, "woT": woT,
            "biasR": biasR, "ones64": np.ones((1, 128), np.float32),
            "e4": e4, "pswap": psw,
        })

    res = run_bass_kernel_spmd(nc, in_maps, core_ids=list(range(NCORES)))
    kernel._last_result = res

    out = np.empty((B, SQ, D), np.float32)
    for core in range(NCORES):
        b, half = core // 2, core % 2
        out[b, half * SQL:(half + 1) * SQL, :] = res.results[core]["outT"].T
    return out


# revision 30
# speedup vs baseline: 1.2277x; 1.0102x over previous
"""Cross-attention (RMSNorm + QKV proj + 2D RoPE + SDPA + out-proj) on 8
Trainium2 NeuronCores.

Sharding: 8 cores = 4 batches x 2 query-halves. Each core computes the full
KV projection for its batch (duplicated across the 2 cores sharing a batch)
and attention + output projection for its 512 query rows. No collectives.

On-device layout is feature-major: activations live as [feature, seq] with
features on SBUF partitions. Host pre-transposes inputs and weights (fp16)
so every linear layer is a plain lhsT.T @ rhs PE matmul at full rate. Head
dims are de-interleaved (even rot dims then odd rot dims per head) so RoPE's
pair rotation becomes a 32-partition block swap plus two fused
(bias-add)*table multiplies against host-precomputed sin/cos tables (fp16,
sign folded into the sin rows).

Every matmul keeps the full 128x128 array busy so the PE clock-gate (HAM)
stays at full rate: attention operands are bf16 with K for both heads of a
chunk packed on the contraction dim and the per-head Q zero-padded on its
unused 64 partitions; V tiles are padded to 128 columns (ones column at 64
for the free softmax denominator, zeros above). Projections run m-outer
with all six weight chunks resident so each PSUM accumulator retires after
six back-to-back matmuls. RMSNorm's rsqrt is exp(-0.5*ln(x)) so one ACT
table set serves the whole kernel; softmax skips max-subtraction and each
exp covers a 3-bank PSUM group (N=1536). The per-head denominator
reciprocal runs straight off PSUM with a DRAM-broadcast roundtrip, and the
output projection is a dense c-outer tail.
"""

import numpy as np

B, SQ, SK, D = 4, 1024, 1024, 768
H, HD = 12, 64
DC = D // 128          # 6 feature chunks
SQL = SQ // 2          # 512 query rows per core
SKC = SK // 128        # 8 key chunks
EPS = 1e-5
NCORES = 8

_cache = {}


# ---------------------------------------------------------------------------
# compiler workarounds
# ---------------------------------------------------------------------------

def _apply_patches():
    """This walrus build allows only ONE sync-wait command per instruction.
    (a) split the Tile kernel-tail drain into one drain per waited proc;
    (b) post-process the BIR JSON, moving excess waits onto same-engine NoOps
    inserted immediately before the over-subscribed instruction."""
    import json
    import concourse.tile as tile
    import concourse.bass as cbass
    from concourse.vector_clock import ScopedClock, VectorClock

    if getattr(cbass.Bass, "_wait_split_patched", False):
        return

    def _drain_and_barrier(self, tick_clock, wait_clock):
        gc = tick_clock.global_clock
        try:
            vec = gc[None]
        except Exception:
            vec = gc
        n = len(vec)
        for p in [i for i in range(n) if vec[i] > 0]:
            sub = [0] * n
            sub[p] = vec[p]
            inst = self.nc.sync.drain()
            wait_clock.add_sem_waits(inst.ins, ScopedClock({None: VectorClock(sub)}))
        self.nc.all_engine_barrier()
        assert self.sems is not None
        popped = self.nc._tile_sem_poison_stack.pop()
        assert popped is self._sem_poison
        self.nc.clear_and_free_semaphores(list(self.sems.allocated().values()))
        self.nc.all_engine_barrier()

    tile.TileContext._drain_and_barrier = _drain_and_barrier

    def _split_waits(bir):
        for f in bir.get("functions", []):
            for blk in f.get("blocks", []):
                insts = blk.get("instructions")
                if not insts:
                    continue
                out = []
                ctr = 0
                for inst in insts:
                    si = inst.get("sync_info")
                    ow = (si or {}).get("on_wait") or []
                    if len(ow) > 1:
                        for w in ow[:-1]:
                            nop = {
                                "name": f"{inst['name']}-ws{ctr}",
                                "opcode": "NoOp",
                                "engine": inst.get("engine"),
                                "ins": [],
                                "outs": [],
                                "sync_info": {"on_wait": [w], "on_update": []},
                            }
                            if "debug" in inst:
                                nop["debug"] = inst["debug"]
                            ctr += 1
                            out.append(nop)
                        si["on_wait"] = [ow[-1]]
                    out.append(inst)
                blk["instructions"] = out
        return bir

    orig = cbass.Bass.to_json_bytes

    def to_json_bytes(self, *a, **kw):
        return json.dumps(_split_waits(json.loads(orig(self, *a, **kw)))).encode()

    cbass.Bass.to_json_bytes = to_json_bytes
    cbass.Bass._wait_split_patched = True


# ---------------------------------------------------------------------------
# device program
# ---------------------------------------------------------------------------

def _build_nc():
    import concourse.bass as bass
    import concourse.tile as tile
    import concourse.mybir as mybir

    F32 = mybir.dt.float32
    F32R = mybir.dt.float32r
    F16 = mybir.dt.float16
    BF16 = mybir.dt.bfloat16
    AF = mybir.ActivationFunctionType
    ALU = mybir.AluOpType

    nc = bass.Bass()

    import concourse.bass as bass_mod

    qT_d = nc.dram_tensor("qT", [128, DC * SQL], F16, kind="ExternalInput")
    kvT_d = nc.dram_tensor("kvT", [128, DC * SK], F16, kind="ExternalInput")
    sqt_d = nc.dram_tensor("sq_tbl", [D, 2, SQL], BF16, kind="ExternalInput")
    skt_d = nc.dram_tensor("sk_tbl", [D, 2, SK], BF16, kind="ExternalInput")
    wq_d = nc.dram_tensor("wqT", [128, DC * D], F16, kind="ExternalInput")
    wk_d = nc.dram_tensor("wkT", [128, DC * D], F16, kind="ExternalInput")
    wv_d = nc.dram_tensor("wvT", [128, DC * D], F16, kind="ExternalInput")
    wo_d = nc.dram_tensor("woT", [128, DC * D], BF16, kind="ExternalInput")
    bias_d = nc.dram_tensor("biasR", [128, 3 * DC], F32, kind="ExternalInput")
    ones64_d = nc.dram_tensor("ones64", [1, 128], F32R, kind="ExternalInput")
    e4_d = nc.dram_tensor("e4", [128, 256], F32R, kind="ExternalInput")
    psw_d = nc.dram_tensor("pswap", [128, 128], BF16, kind="ExternalInput")
    out_d = nc.dram_tensor("outT", [D, SQL], F32, kind="ExternalOutput")


    with tile.TileContext(nc) as tc:
        import contextlib
        ctx = contextlib.ExitStack()
        with ctx:
            persist = ctx.enter_context(tc.tile_pool(name="persist", bufs=1))
            ph1 = ctx.enter_context(tc.tile_pool(name="ph1", bufs=1))
            tmp = ctx.enter_context(tc.tile_pool(name="tmp", bufs=2))
            big = ctx.enter_context(tc.tile_pool(name="big", bufs=2, space="PSUM"))
            pp = ctx.enter_context(tc.tile_pool(name="pp", bufs=2, space="PSUM"))

            # ---- persistent small tensors --------------------------------
            bias_sb = persist.tile([128, 3 * DC], F32)
            ones_sb = persist.tile([128, 128], F16)
            onesc_sb = persist.tile([128, H], BF16)
            ones64 = persist.tile([1, 128], F32R)
            e4_sb = persist.tile([128, 256], F32R)
            psw_sb = persist.tile([128, 128], BF16)
            eps_t = persist.tile([128, 1], F32)
            nc.gpsimd.dma_start(out=bias_sb, in_=bias_d[:, :])
            nc.gpsimd.dma_start(out=ones64, in_=ones64_d[:, :])
            nc.gpsimd.dma_start(out=e4_sb, in_=e4_d[:, :])
            nc.gpsimd.dma_start(out=psw_sb, in_=psw_d[:, :])
            nc.vector.memset(ones_sb, 1.0)
            nc.gpsimd.memset(onesc_sb, 1.0)
            nc.vector.memset(eps_t, EPS)

            # ---- persistent activations ---------------------------------
            qrz = [persist.tile([128, SQL], BF16, name=f"qrz{h}") for h in range(H)]
            krot = [persist.tile([128, SK], BF16, name=f"krot{c}") for c in range(DC)]
            vp = [persist.tile([128, H, 128], BF16, name=f"vp{c}") for c in range(SKC)]
            oTn = [persist.tile([128, SQL], BF16, name=f"oTn{c}") for c in range(DC)]
            out_sb = [persist.tile([128, SQL], F32, name=f"osb{c}") for c in range(DC)]

            # zero the unused halves/padding once (gpsimd engine)
            for h in range(H):
                off = 64 * (h % 2)
                nc.gpsimd.memset(qrz[h][64 - off:128 - off, :], 0.0)
            for kc in range(SKC):
                nc.gpsimd.memset(vp[kc][:, :, HD + 1:128], 0.0)

            # ---- weight + input loads (one DMA per tensor) --------------
            def wfull(dram, dt):
                t = tmp.tile([128, DC * D], dt, tag="wfull", bufs=4, name="wf")
                nc.scalar.dma_start(out=t, in_=dram[:, :])
                return t

            kvT_t = ph1.tile([128, DC * SK], F16, name="kvT")
            qT_t = ph1.tile([128, DC * SQL], F16, name="qT")
            nc.sync.dma_start(out=kvT_t[:, 0:SK], in_=kvT_d[:, 0:SK])
            nc.scalar.dma_start(out=qT_t[:, 0:SQL], in_=qT_d[:, 0:SQL])
            nc.sync.dma_start(out=kvT_t[:, SK:], in_=kvT_d[:, SK:])
            nc.scalar.dma_start(out=qT_t[:, SQL:], in_=qT_d[:, SQL:])
            wk = wfull(wk_d, F16)
            kvT = [kvT_t[:, c * SK:(c + 1) * SK] for c in range(DC)]
            qT = [qT_t[:, c * SQL:(c + 1) * SQL] for c in range(DC)]

            # ---- RMSNorm over features (partition dim) ------------------
            ss = pp.tile([128, SQL], F32, tag="pp")
            for c in range(DC):
                sq = tmp.tile([128, SQL], F16, tag="sq", bufs=2, name="sq")
                nc.vector.tensor_mul(out=sq, in0=qT[c], in1=qT[c])
                nc.tensor.matmul(ss, ones_sb, sq, start=(c == 0), stop=(c == DC - 1))
            lnv = tmp.tile([128, SQL], F32, tag="lnv", name="lnv")
            nc.scalar.activation(out=lnv, in_=ss, func=AF.Ln, bias=eps_t,
                                 scale=1.0 / D)
            rstd = tmp.tile([128, SQL], F32, tag="rstd", name="rstd")
            nc.scalar.activation(out=rstd, in_=lnv, func=AF.Exp, scale=-0.5)
            for c in range(DC):
                nc.vector.tensor_mul(out=qT[c], in0=qT[c], in1=rstd)

            # ---- projection + RoPE (m-outer, resident weights) ----------
            def proj_rope(wt, xs, xsl, bcol, tbl_dram, stride2, coff,
                          emit_rot):
                pk = [big.tile([128, 3 * SQL], F32, tag="big", name=f"pk{j}")
                      for j in range(2)]
                for m in range(DC):
                    j, mm = divmod(m, 3)
                    sl = pk[j][:, mm * SQL:(mm + 1) * SQL]
                    for c in range(DC):
                        nc.tensor.matmul(
                            sl, wt[:, c * D + m * 128:c * D + (m + 1) * 128],
                            xs[c][:, xsl],
                            start=(c == 0), stop=(c == DC - 1))
                    tbl = tmp.tile([128, 2 * SQL], BF16, tag="tbl", bufs=4,
                                   name="tbl")
                    t0 = tbl_dram[:, :, :]
                    nc.sync.dma_start(
                        out=tbl.rearrange("p (t s) -> p t s", t=2),
                        in_=bass_mod.AP(
                            tensor=t0.tensor,
                            offset=m * 128 * 2 * stride2 + coff,
                            ap=[[2 * stride2, 128], [stride2, 2], [1, SQL]]))
                    sin_t, cos_t = tbl[:, 0:SQL], tbl[:, SQL:2 * SQL]
                    # biased projection to SBUF, then a 32-block swapped copy
                    kp = tmp.tile([128, SQL], BF16, tag="kp", bufs=4, name="kp")
                    nc.scalar.activation(
                        out=kp, in_=sl, func=AF.Identity,
                        bias=bias_sb[:, bcol + m:bcol + m + 1])
                    t1p = pp.tile([128, SQL], F32, tag="pp", name="t1p")
                    nc.tensor.matmul(t1p, psw_sb, kp, start=True, stop=True)
                    t1 = tmp.tile([128, SQL], BF16, tag="sw", bufs=4, name="t1")
                    nc.vector.tensor_mul(out=t1, in0=t1p, in1=sin_t)
                    emit_rot(m, kp, t1, cos_t)

            def rot_k(hs):
                def emit(m, kp, t1, cos_t):
                    dst = krot[m][:, hs]
                    nc.vector.tensor_mul(out=dst, in0=kp, in1=cos_t)
                    nc.vector.tensor_add(out=dst, in0=dst, in1=t1)
                return emit

            def rot_q(m, kp, t1, cos_t):
                for h, pr in ((2 * m, slice(0, 64)), (2 * m + 1, slice(64, 128))):
                    dst = qrz[h][pr, :]
                    nc.vector.tensor_mul(out=dst, in0=kp[pr, :],
                                         in1=cos_t[pr, :])
                    nc.vector.tensor_add(out=dst, in0=dst, in1=t1[pr, :])

            wq = wfull(wq_d, F16)
            wv = wfull(wv_d, F16)
            proj_rope(wk, kvT, slice(0, SQL), DC, skt_d, SK,
                      0, rot_k(slice(0, SQL)))
            proj_rope(wk, kvT, slice(SQL, SK), DC, skt_d, SK,
                      SQL, rot_k(slice(SQL, SK)))
            proj_rope(wq, qT, slice(0, SQL), 0, sqt_d, SQL,
                      0, rot_q)

            # ---- V projection (row-major, ones column at 64); emission
            # of the kc chunks is interleaved with head-0 attention so
            # softmax work starts while V is still projecting ------------
            def v_chunk(kc):
                ksl = slice(kc * 128, (kc + 1) * 128)
                pv = big.tile([128, 3 * SQL], F32, tag="big", name="pv")
                for c in range(DC):
                    nc.tensor.matmul(pv[:, 0:512], kvT[c][:, ksl],
                                     wv[:, c * D:c * D + 512],
                                     start=(c == 0), stop=(c == DC - 1))
                    nc.tensor.matmul(pv[:, 512:768], kvT[c][:, ksl],
                                     wv[:, c * D + 512:c * D + 768],
                                     start=(c == 0), stop=(c == DC - 1))
                nc.vector.tensor_copy(
                    out=vp[kc][:, 0:8, 0:HD],
                    in_=pv[:, 0:512].rearrange("p (h d) -> p h d", h=8))
                nc.vector.tensor_copy(
                    out=vp[kc][:, 8:12, 0:HD],
                    in_=pv[:, 512:768].rearrange("p (h d) -> p h d", h=4))
                nc.gpsimd.tensor_copy(out=vp[kc][:, :, HD], in_=onesc_sb)

            for kc in range(3):
                v_chunk(kc)

            # ---- attention + pipelined per-head normalization -----------
            GROUPS = [(0, 3), (3, 6), (6, 8)]
            den4 = None
            for h in range(H):
                mh, off = h // 2, 64 * (h % 2)
                if h % 4 == 0:
                    den4 = tmp.tile([128, SQL], F32, tag="den4", bufs=2,
                                    name="den4")
                    nc.gpsimd.memset(den4, 1.0)
                po = pp.tile([128, SQL], F32, tag="pp", name="po")
                for gi, (k0, k1) in enumerate(GROUPS):
                    w = (k1 - k0) * SQL
                    sc = big.tile([128, 3 * SQL], F32, tag="big", name="sc")
                    for i, kc in enumerate(range(k0, k1)):
                        nc.tensor.matmul(
                            sc[:, i * SQL:(i + 1) * SQL],
                            krot[mh][:, kc * 128:(kc + 1) * 128],
                            qrz[h], start=True, stop=True)
                    if h == 0 and gi == 0:
                        for kc2 in range(3, 6):
                            v_chunk(kc2)
                    if h == 0 and gi == 1:
                        for kc2 in range(6, 8):
                            v_chunk(kc2)
                    ex = tmp.tile([128, 3 * SQL], BF16, tag="ex", bufs=3,
                                  name="ex")
                    nc.scalar.activation(out=ex[:, :w], in_=sc[:, :w],
                                         func=AF.Exp, scale=1.0 / 8.0)
                    for i, kc in enumerate(range(k0, k1)):
                        nc.tensor.matmul(po, vp[kc][:, h, :],
                                         ex[:, i * SQL:(i + 1) * SQL],
                                         start=(kc == 0), stop=(kc == SKC - 1))
                # stash unnormalized O + the denominator row; po retires fast
                b4 = 32 * (h % 4)
                nc.vector.tensor_copy(out=den4[b4:b4 + 1, :],
                                      in_=po[64:65, :])
                nc.vector.tensor_copy(out=oTn[mh][off:off + 64, :],
                                      in_=po[0:64, :])
                if h % 4 == 3:
                    # batched reciprocal on ACT (exp(-ln x), free-dim bound),
                    # then per-pair partition-broadcast via a PE matmul
                    ln4 = tmp.tile([128, SQL], F32, tag="ln4", bufs=2,
                                   name="ln4")
                    nc.scalar.activation(out=ln4, in_=den4, func=AF.Ln)
                    rd4 = tmp.tile([128, SQL], F32R, tag="rd4", bufs=2,
                                   name="rd4")
                    nc.scalar.activation(out=rd4, in_=ln4, func=AF.Exp,
                                         scale=-1.0)
                    for p in range(2):
                        mh2 = (h // 4) * 2 + p
                        rbb = pp.tile([128, SQL], F32, tag="pp", name="rbb")
                        nc.tensor.matmul(rbb, e4_sb[:, p * 128:(p + 1) * 128],
                                         rd4, start=True, stop=True)
                        rbs = tmp.tile([128, SQL], F32, tag="rb", bufs=2,
                                       name="rbs")
                        nc.vector.tensor_copy(out=rbs, in_=rbb)
                        nc.vector.tensor_mul(out=oTn[mh2], in0=oTn[mh2],
                                             in1=rbs)

            # ---- output projection tail (c-outer, 2 psum accumulators) --
            wo = wfull(wo_d, BF16)
            po6 = [big.tile([128, 3 * SQL], F32, tag="big", name=f"po6_{j}")
                   for j in range(2)]
            for c in range(DC):
                for j in range(2):
                    for mm in range(3):
                        m = 3 * j + mm
                        nc.tensor.matmul(
                            po6[j][:, mm * SQL:(mm + 1) * SQL],
                            wo[:, c * D + m * 128:c * D + (m + 1) * 128],
                            oTn[c], start=(c == 0), stop=(c == DC - 1))
            for m in range(DC):
                j, mm = divmod(m, 3)
                nc.vector.tensor_scalar_add(
                    out=out_sb[m], in0=po6[j][:, mm * SQL:(mm + 1) * SQL],
                    scalar1=bias_sb[:, 2 * DC + m:2 * DC + m + 1])
                nc.sync.dma_start(out=out_d[m * 128:(m + 1) * 128, :],
                                  in_=out_sb[m])

    return nc


# ---------------------------------------------------------------------------
# host wrapper
# ---------------------------------------------------------------------------

def _rope_tables(pos, freqs):
    """pos [S,2] int, freqs [2,H,32] -> signed-sin and cos tables [D,S] fp16,
    rows in de-interleaved head-dim order (pair j at rows h*64+j / h*64+32+j,
    first-half sin rows negated so rot = x*cos + swap(x)*sin)."""
    ang = np.einsum('sd,dhj->hjs', pos.astype(np.float64),
                    freqs.astype(np.float64))          # [H,32,S]
    s, c = np.sin(ang), np.cos(ang)
    sin_full = np.concatenate([-s, s], axis=1).reshape(D, -1)
    cos_full = np.concatenate([c, c], axis=1).reshape(D, -1)
    import ml_dtypes
    bf = ml_dtypes.bfloat16
    return sin_full.astype(bf), cos_full.astype(bf)


def kernel(q, kv, posq, posk, w_norm, w_q, b_q, w_kv, b_kv, w_out, b_out, freqs):
    _apply_patches()
    from concourse.bass_utils import run_bass_kernel_spmd
    import ml_dtypes

    bf16 = ml_dtypes.bfloat16

    q = np.asarray(q, np.float32)
    kv = np.asarray(kv, np.float32)
    posq_np = np.asarray(posq)
    posk_np = np.asarray(posk)
    w_norm = np.asarray(w_norm, np.float32)
    w_q = np.asarray(w_q, np.float32)
    b_q = np.asarray(b_q, np.float32)
    w_kv = np.asarray(w_kv, np.float32)
    b_kv = np.asarray(b_kv, np.float32)
    w_out = np.asarray(w_out, np.float32)
    b_out = np.asarray(b_out, np.float32)
    freqs = np.asarray(freqs, np.float32)

    # de-interleave head dims: new j<32 -> old 2j (even), j>=32 -> old 2(j-32)+1
    perm = np.empty(D, np.int64)
    for h in range(H):
        for j in range(HD):
            perm[h * HD + j] = h * HD + (2 * j if j < 32 else 2 * (j - 32) + 1)

    def sblay(w, dt):
        # [D, W] feature-major -> [128, DC*W] SBUF tile layout
        Wd = w.shape[1]
        return np.ascontiguousarray(
            w.reshape(DC, 128, Wd).transpose(1, 0, 2).reshape(128, DC * Wd)
        ).astype(dt)

    wqT = sblay((w_q[perm, :] * w_norm[None, :]).T, np.float16)
    wkT = sblay(w_kv[:D][perm, :].T, np.float16)
    wvT = sblay(w_kv[D:].T, np.float16)
    woT = sblay(w_out.T, bf16)
    bqR = np.ascontiguousarray(b_q[perm].reshape(DC, 128).T)
    bkR = np.ascontiguousarray(b_kv[:D][perm].reshape(DC, 128).T)
    bo_eff = b_out + w_out @ b_kv[D:]          # fold V bias (softmax sums to 1)
    boR = np.ascontiguousarray(bo_eff.reshape(DC, 128).T)

    biasR = np.ascontiguousarray(
        np.concatenate([bqR, bkR, boR], axis=1)).astype(np.float32)
    psw = np.zeros((128, 128), np.float32)
    for i in range(128):
        j = (i % 64)
        partner = (i - i % 64) + (j + 32 if j < 32 else j - 32)
        psw[partner, i] = 1.0
    psw = psw.astype(bf16)
    e4 = np.zeros((128, 256), np.float32)
    e4[0, 0:64] = 1.0
    e4[32, 64:128] = 1.0
    e4[64, 128:192] = 1.0
    e4[96, 192:256] = 1.0

    if "nc" not in _cache:
        _cache["nc"] = _build_nc()
    nc = _cache["nc"]

    in_maps = []
    for core in range(NCORES):
        b, half = core // 2, core % 2
        qs = slice(half * SQL, (half + 1) * SQL)
        sinq, cosq = _rope_tables(posq_np[b], freqs)
        sink, cosk = _rope_tables(posk_np[b], freqs)
        in_maps.append({
            "qT": sblay(q[b, qs, :].T, np.float16),
            "kvT": sblay(kv[b].T, np.float16),
            "sq_tbl": np.ascontiguousarray(
                np.stack([sinq[:, qs], cosq[:, qs]], axis=1)),
            "sk_tbl": np.ascontiguousarray(np.stack([sink, cosk], axis=1)),
            "wqT": wqT, "wkT": wkT, "wvT": wvT, "woT": woT,
            "biasR": biasR, "ones64": np.ones((1, 128), np.float32),
            "e4": e4, "pswap": psw,
        })

    res = run_bass_kernel_spmd(nc, in_maps, core_ids=list(range(NCORES)))
    kernel._last_result = res

    out = np.empty((B, SQ, D), np.float32)
    for core in range(NCORES):
        b, half = core // 2, core % 2
        out[b, half * SQL:(half + 1) * SQL, :] = res.results[core]["outT"].T
    return out
